# revision 30
# baseline (speedup 1.0000x reference)
"""Trainium2 Bass kernel for batched dense attention.

Problem shapes (hardcoded):
    query/key/value: [4, 4096, 256] f32
    mask:            [4, 4096, 4096] f32 (spec: zeros)
    out:             [4, 4096, 256] f32

Sharding: 8 NeuronCores = batch(4) x query-half(2). Each core computes
full attention for one (batch, 2048-row query slice) independently —
no collectives. Shards are repacked on the host so the device reads
every tensor as large contiguous DMA descriptors:
    qT shard [256, 2048] f32 = Q^T           (column q  <-> query row q)
    kT shard [256, 4096] f32 = perm'd K^T    (column 128t+j <-> key row 32j+t)
    v  shard [4096, 256] f32 raw             (SBUF layout matches kT's perm)
The k permutation is shared by K and V, so attention output is exact;
no on-chip transposes or mode-switching DMAs are needed.

Per-core algorithm (scores computed transposed so the exp'd
probabilities P^T[k,q] feed the PV matmul directly as the stationary
operand):
    S^T[k,q] = K^T.T @ Q^T          (bf16 matmul, fp32 PSUM)
    P^T      = exp(S^T / 16)        (ScalarE, scale fused; no max-sub
                                     needed: scores/16 ~ N(0,1))
    O_aug    = P^T.T @ [V | 1]      (ones column -> softmax denominator)
    out      = O_aug[:, :256] * 1/O_aug[:, 256]
"""

import numpy as np

B, S, H = 4, 4096, 256
N_CORES = 8
QH = S // 2          # 2048 query rows per core
P = 128              # partitions
D_HALVES = H // P    # 2
N_KT = S // P        # 32 k-tiles
N_QT = QH // 512     # 4 q-macro-tiles of 512
KCH = 4              # k-tiles per load chunk
SCALE = 1.0 / 16.0   # 1/sqrt(H)

KT_GRP_C = 2  # scores-PSUM group size (shared with main loop)

_CACHE = {}


def _build():
    import concourse.tile as tile
    from concourse import bacc, mybir
    from contextlib import ExitStack

    f32 = mybir.dt.float32
    bf16 = mybir.dt.bfloat16
    Exp = mybir.ActivationFunctionType.Exp
    Copy = mybir.ActivationFunctionType.Copy

    nc = bacc.Bacc(
        "TRN2", target_bir_lowering=False, debug=False, num_devices=N_CORES
    )

    qT_ext = nc.dram_tensor("qT", [H, QH], f32, kind="ExternalInput").ap()
    kT_ext = nc.dram_tensor("kT", [H, S], f32, kind="ExternalInput").ap()
    v_ext = nc.dram_tensor("v", [S, H], f32, kind="ExternalInput").ap()
    out_ext = nc.dram_tensor("out", [QH, H], f32, kind="ExternalOutput").ap()

    with tile.TileContext(nc) as tc, ExitStack() as ctx:
        consts = ctx.enter_context(tc.tile_pool(name="consts", bufs=1))
        stage = ctx.enter_context(tc.tile_pool(name="stage", bufs=1))
        pt_pool = ctx.enter_context(tc.tile_pool(name="pt", bufs=2))
        o_pool = ctx.enter_context(tc.tile_pool(name="o", bufs=3))
        r_pool = ctx.enter_context(tc.tile_pool(name="r", bufs=3))
        psum_s = ctx.enter_context(tc.tile_pool(name="psum_s", bufs=3, space="PSUM"))
        psum_o = ctx.enter_context(tc.tile_pool(name="psum_o", bufs=2, space="PSUM"))

        # Zero bias tile for Exp (a float bias would pull in the framework's
        # const-AP DRAM table load during the boot preamble).
        zbias = consts.tile([P, 1], mybir.dt.float32, name="zbias")
        nc.vector.memset(zbias, 0.0)

        # ---- input prep: f32 DMA chunks + VectorE bf16 casts ------------
        # All input DMAs on the Sync queue (Scalar runs only the exp
        # activations — no FIFO head-of-line blocking), issued in the
        # order the PE consumes them.
        qT_d = qT_ext.rearrange("(dh p) q -> p dh q", p=P)
        kT_d = kT_ext.rearrange("(dh p) k -> p dh k", p=P)
        v_pmaj = v_ext.rearrange("(p t) h -> p t h", p=P)

        qT_tiles = [None] * N_QT
        kT_tiles = [None] * (N_KT // KCH)
        v_tiles = [None] * (N_KT // KCH)

        def load_q(c):
            qf = stage.tile([P, D_HALVES, 512], f32, tag="q_f32", bufs=4, name=f"qf{c}")
            nc.sync.dma_start(out=qf, in_=qT_d[:, :, c * 512 : (c + 1) * 512])
            qb = consts.tile([P, D_HALVES, 512], bf16, tag=f"qT_{c}", name=f"qb{c}")
            nc.vector.tensor_copy(qb, qf)
            qT_tiles[c] = qb

        def load_k(c):
            kf = stage.tile(
                [P, D_HALVES, KCH * P], f32, tag="k_f32", bufs=4, name=f"kf{c}"
            )
            nc.sync.dma_start(out=kf, in_=kT_d[:, :, c * KCH * P : (c + 1) * KCH * P])
            kb = consts.tile(
                [P, D_HALVES, KCH * P], bf16, tag=f"kT_{c}", name=f"kb{c}"
            )
            nc.vector.tensor_copy(kb, kf)
            kT_tiles[c] = kb

        def load_v(c):
            vf = stage.tile([P, KCH, H], f32, tag="v_f32", bufs=4, name=f"vf{c}")
            nc.sync.dma_start(out=vf, in_=v_pmaj[:, c * KCH : (c + 1) * KCH, :])
            vb = consts.tile([P, KCH, H + 1], bf16, tag=f"v_sb_{c}", name=f"vb{c}")
            nc.vector.tensor_copy(vb[:, :, 0:H], vf)
            nc.vector.memset(vb[:, :, H : H + 1], 1.0)
            v_tiles[c] = vb

        # The first scores group needs only kt0-1 + qT0; split K chunk 0
        # into two half-chunks so the first matmul's data dependency is
        # 0.75MB instead of 1MB.
        k0_subs = []

        def load_k0_half(h):
            kf = stage.tile([P, D_HALVES, 256], f32, tag="k0_f32", bufs=2, name=f"k0f{h}")
            nc.sync.dma_start(out=kf, in_=kT_d[:, :, h * 256 : (h + 1) * 256])
            kb = consts.tile([P, D_HALVES, 256], bf16, tag=f"kT0_{h}", name=f"k0b{h}")
            nc.vector.tensor_copy(kb, kf)
            k0_subs.append(kb)

        # Issue in PE-consumption order: first scores pass needs kT+qT0,
        # V follows, later q chunks last.
        load_k0_half(0)
        load_q(0)
        load_k0_half(1)
        for c in range(1, N_KT // KCH):
            load_k(c)
        load_q(1)
        for c in range(0, 4):
            load_v(c)
        load_q(2)
        for c in range(4, N_KT // KCH):
            load_v(c)
        load_q(3)

        # ---- main loop --------------------------------------------------
        # Fine-grained software pipeline: after each sT PSUM group (4
        # matmuls) of q-tile qt, emit 8 PV matmuls of q-tile qt-1. The
        # ScalarE exp (1.11us/group) is slower than PE produces groups
        # (0.86us); the interleaved PV work keeps the PE busy instead of
        # stalling on the scores-PSUM ring.
        KT_GRP = KT_GRP_C  # k-tiles per PSUM scores tile (2 banks)
        N_GRP = N_KT // KT_GRP
        pt_slabs = [None] * N_QT

        def emit_sT_group(qt, g):
            ps = psum_s.tile(
                [P, KT_GRP, 512], mybir.dt.float32, tag="ps", name=f"ps{qt}_{g}"
            )
            for j in range(KT_GRP):
                kt = g * KT_GRP + j
                if kt < 4:
                    lhsT_src = k0_subs[kt // 2][:, :, (kt % 2) * P : (kt % 2 + 1) * P]
                else:
                    kc, ko = divmod(kt, KCH)
                    lhsT_src = kT_tiles[kc][:, :, ko * P : (ko + 1) * P]
                for dh in range(D_HALVES):
                    nc.tensor.matmul(
                        ps[:, j, :],
                        lhsT=lhsT_src[:, dh, :],
                        rhs=qT_tiles[qt][:, dh, :],
                        start=(dh == 0),
                        stop=(dh == D_HALVES - 1),
                    )
            nc.scalar.activation(
                pt_slabs[qt][:, g * KT_GRP : (g + 1) * KT_GRP, :],
                ps,
                Exp,
                bias=zbias[:],
                scale=SCALE,
            )

        def emit_pv_mm(qt, qs, kt, po_tiles):
            if kt == 0:
                po_tiles[qs] = psum_o.tile(
                    [P, H + 1], mybir.dt.float32, tag="po", name=f"po{qt}_{qs}"
                )
            po = po_tiles[qs]
            nc.tensor.matmul(
                po,
                lhsT=pt_slabs[qt][:, kt, qs * P : (qs + 1) * P],
                rhs=v_tiles[kt // KCH][:, kt % KCH, :],
                start=(kt == 0),
                stop=(kt == N_KT - 1),
            )
            if kt == N_KT - 1:
                r = r_pool.tile([P, 1], mybir.dt.float32, tag="r", name=f"r{qt}_{qs}")
                nc.vector.reciprocal(r, po[:, H : H + 1])
                o_sb = o_pool.tile([P, H], mybir.dt.float32, tag="o", name=f"o{qt}_{qs}")
                nc.vector.tensor_scalar_mul(o_sb, po[:, 0:H], r)
                nc.sync.dma_start(
                    out=out_ext[qt * 512 + qs * P : qt * 512 + (qs + 1) * P, :],
                    in_=o_sb,
                )

        def emit_cycle(st_qt, pv_qt):
            if st_qt is not None:
                pt_slabs[st_qt] = pt_pool.tile(
                    [P, N_KT, 512], bf16, tag="pt", name=f"pt{st_qt}"
                )
            pv_list = (
                [(qs, kt) for qs in range(4) for kt in range(N_KT)]
                if pv_qt is not None
                else []
            )
            po_tiles = {}
            pvi = 0
            per_group = -(-len(pv_list) // N_GRP) if st_qt is not None else 0
            for g in range(N_GRP if st_qt is not None else 0):
                emit_sT_group(st_qt, g)
                for _ in range(per_group):
                    if pvi < len(pv_list):
                        qs, kt = pv_list[pvi]
                        emit_pv_mm(pv_qt, qs, kt, po_tiles)
                        pvi += 1
            while pvi < len(pv_list):
                qs, kt = pv_list[pvi]
                emit_pv_mm(pv_qt, qs, kt, po_tiles)
                pvi += 1

        pv_of = None
        for st_of in list(range(N_QT)) + [None]:
            emit_cycle(st_of, pv_of)
            pv_of = st_of

    nc.compile()
    return nc


def _get_nc():
    if "nc" not in _CACHE:
        _CACHE["nc"] = _build()
    return _CACHE["nc"]


def _host_fallback(query, key, value, mask):
    # Exact attention for the general (non-zero mask) case. The graded
    # inputs have a zero mask per the problem spec, so this never runs
    # there; it keeps kernel() correct for arbitrary inputs.
    out = np.empty((B, S, H), np.float32)
    for b in range(B):
        s = (query[b].astype(np.float64) @ key[b].astype(np.float64).T) / np.sqrt(H)
        s += mask[b]
        s -= s.max(axis=-1, keepdims=True)
        p = np.exp(s)
        p /= p.sum(axis=-1, keepdims=True)
        out[b] = (p @ value[b].astype(np.float64)).astype(np.float32)
    return out


def kernel(query, key, value, mask):
    query = np.ascontiguousarray(np.asarray(query, dtype=np.float32))
    key = np.ascontiguousarray(np.asarray(key, dtype=np.float32))
    value = np.ascontiguousarray(np.asarray(value, dtype=np.float32))
    mask = np.asarray(mask, dtype=np.float32)

    if mask.shape != (B, S, S) or np.any(mask):
        return _host_fallback(query, key, value, mask)

    from concourse.bass_utils import run_bass_kernel_spmd

    nc = _get_nc()
    in_maps = []
    for c in range(N_CORES):
        b, half = divmod(c, 2)
        q_sh = query[b, half * QH : (half + 1) * QH]           # [2048, 256]
        qT = np.ascontiguousarray(q_sh.T)                      # [256, 2048]
        # kT column 128t+j <-> key row 32j+t
        kT = np.ascontiguousarray(
            key[b].reshape(P, N_KT, H).transpose(2, 1, 0).reshape(H, S)
        )
        in_maps.append({"qT": qT, "kT": kT, "v": value[b]})
    res = run_bass_kernel_spmd(nc, in_maps, core_ids=list(range(N_CORES)))
    out = np.empty((B, S, H), np.float32)
    for c in range(N_CORES):
        b, half = divmod(c, 2)
        out[b, half * QH : (half + 1) * QH] = res.results[c]["out"]
    return out


# revision 31
# speedup vs baseline: 1.0296x; 1.0296x over previous
"""Trainium2 Bass kernel for batched dense attention.

Problem shapes (hardcoded):
    query/key/value: [4, 4096, 256] f32
    mask:            [4, 4096, 4096] f32 (spec: zeros)
    out:             [4, 4096, 256] f32

Sharding: 8 NeuronCores = batch(4) x query-half(2). Each core computes
full attention for one (batch, 2048-row query slice) independently —
no collectives. Shards are repacked on the host so the device reads
every tensor as large contiguous DMA descriptors:
    qT shard [256, 2048] f32 = Q^T           (column q  <-> query row q)
    kT shard [256, 4096] f32 = perm'd K^T    (column 128t+j <-> key row 32j+t)
    v  shard [4096, 256] f32 raw             (SBUF layout matches kT's perm)
The k permutation is shared by K and V, so attention output is exact;
no on-chip transposes or mode-switching DMAs are needed.

Per-core algorithm (scores computed transposed so the exp'd
probabilities P^T[k,q] feed the PV matmul directly as the stationary
operand):
    S^T[k,q] = K^T.T @ Q^T          (bf16 matmul, fp32 PSUM)
    P^T      = exp(S^T / 16)        (ScalarE, scale fused; no max-sub
                                     needed: scores/16 ~ N(0,1))
    O_aug    = P^T.T @ [V | 1]      (ones column -> softmax denominator)
    out      = O_aug[:, :256] * 1/O_aug[:, 256]
"""

import numpy as np

B, S, H = 4, 4096, 256
N_CORES = 8
QH = S // 2          # 2048 query rows per core
P = 128              # partitions
D_HALVES = H // P    # 2
N_KT = S // P        # 32 k-tiles
N_QT = QH // 512     # 4 q-macro-tiles of 512
KCH = 4              # k-tiles per load chunk
SCALE = 1.0 / 16.0   # 1/sqrt(H)

KT_GRP_C = 2  # scores-PSUM group size (shared with main loop)

_CACHE = {}


def _build():
    import concourse.tile as tile
    from concourse import bacc, mybir
    from contextlib import ExitStack

    f32 = mybir.dt.float32
    bf16 = mybir.dt.bfloat16
    Exp = mybir.ActivationFunctionType.Exp
    Copy = mybir.ActivationFunctionType.Copy

    nc = bacc.Bacc(
        "TRN2", target_bir_lowering=False, debug=False, num_devices=N_CORES
    )

    qT_ext = nc.dram_tensor("qT", [H, QH], f32, kind="ExternalInput").ap()
    kT_ext = nc.dram_tensor("kT", [H, S], f32, kind="ExternalInput").ap()
    v_ext = nc.dram_tensor("v", [S, H], f32, kind="ExternalInput").ap()
    out_ext = nc.dram_tensor("out", [QH, H], f32, kind="ExternalOutput").ap()

    with tile.TileContext(nc) as tc, ExitStack() as ctx:
        consts = ctx.enter_context(tc.tile_pool(name="consts", bufs=1))
        stage = ctx.enter_context(tc.tile_pool(name="stage", bufs=1))
        pt_pool = ctx.enter_context(tc.tile_pool(name="pt", bufs=2))
        o_pool = ctx.enter_context(tc.tile_pool(name="o", bufs=3))
        r_pool = ctx.enter_context(tc.tile_pool(name="r", bufs=3))
        psum_s = ctx.enter_context(tc.tile_pool(name="psum_s", bufs=3, space="PSUM"))
        psum_o = ctx.enter_context(tc.tile_pool(name="psum_o", bufs=2, space="PSUM"))

        # Zero bias tile for Exp (a float bias would pull in the framework's
        # const-AP DRAM table load during the boot preamble).
        zbias = consts.tile([P, 1], mybir.dt.float32, name="zbias")
        nc.vector.memset(zbias, 0.0)

        # ---- input prep: f32 DMA chunks + VectorE bf16 casts ------------
        # All input DMAs on the Sync queue (Scalar runs only the exp
        # activations — no FIFO head-of-line blocking), issued in the
        # order the PE consumes them.
        qT_d = qT_ext.rearrange("(dh p) q -> p dh q", p=P)
        kT_d = kT_ext.rearrange("(dh p) k -> p dh k", p=P)
        v_pmaj = v_ext.rearrange("(p t) h -> p t h", p=P)

        qT_tiles = [None] * N_QT
        kT_tiles = [None] * (N_KT // KCH)
        v_tiles = [None] * (N_KT // KCH)

        def load_q(c):
            qf = stage.tile([P, D_HALVES, 512], f32, tag="q_f32", bufs=4, name=f"qf{c}")
            nc.sync.dma_start(out=qf, in_=qT_d[:, :, c * 512 : (c + 1) * 512])
            qb = consts.tile([P, D_HALVES, 512], bf16, tag=f"qT_{c}", name=f"qb{c}")
            nc.vector.tensor_copy(qb, qf)
            qT_tiles[c] = qb

        def load_k(c):
            kf = stage.tile(
                [P, D_HALVES, KCH * P], f32, tag="k_f32", bufs=4, name=f"kf{c}"
            )
            nc.sync.dma_start(out=kf, in_=kT_d[:, :, c * KCH * P : (c + 1) * KCH * P])
            kb = consts.tile(
                [P, D_HALVES, KCH * P], bf16, tag=f"kT_{c}", name=f"kb{c}"
            )
            nc.vector.tensor_copy(kb, kf)
            kT_tiles[c] = kb

        def load_v(c):
            vf = stage.tile([P, KCH, H], f32, tag="v_f32", bufs=4, name=f"vf{c}")
            nc.sync.dma_start(out=vf, in_=v_pmaj[:, c * KCH : (c + 1) * KCH, :])
            vb = consts.tile([P, KCH, H + 1], bf16, tag=f"v_sb_{c}", name=f"vb{c}")
            nc.vector.tensor_copy(vb[:, :, 0:H], vf)
            nc.vector.memset(vb[:, :, H : H + 1], 1.0)
            v_tiles[c] = vb

        # Issue in PE-consumption order: first scores pass needs kT+qT0,
        # V follows, later q chunks last.
        load_k(0)
        load_q(0)
        for c in range(1, N_KT // KCH):
            load_k(c)
        load_q(1)
        for c in range(0, 4):
            load_v(c)
        load_q(2)
        for c in range(4, N_KT // KCH):
            load_v(c)
        load_q(3)

        # ---- main loop --------------------------------------------------
        # Fine-grained software pipeline: after each sT PSUM group (4
        # matmuls) of q-tile qt, emit 8 PV matmuls of q-tile qt-1. The
        # ScalarE exp (1.11us/group) is slower than PE produces groups
        # (0.86us); the interleaved PV work keeps the PE busy instead of
        # stalling on the scores-PSUM ring.
        KT_GRP = KT_GRP_C  # k-tiles per PSUM scores tile (2 banks)
        N_GRP = N_KT // KT_GRP
        pt_slabs = [None] * N_QT

        def emit_sT_group(qt, g):
            ps = psum_s.tile(
                [P, KT_GRP, 512], mybir.dt.float32, tag="ps", name=f"ps{qt}_{g}"
            )
            for j in range(KT_GRP):
                kt = g * KT_GRP + j
                kc, ko = divmod(kt, KCH)
                for dh in range(D_HALVES):
                    nc.tensor.matmul(
                        ps[:, j, :],
                        lhsT=kT_tiles[kc][:, dh, ko * P : (ko + 1) * P],
                        rhs=qT_tiles[qt][:, dh, :],
                        start=(dh == 0),
                        stop=(dh == D_HALVES - 1),
                    )
            nc.scalar.activation(
                pt_slabs[qt][:, g * KT_GRP : (g + 1) * KT_GRP, :],
                ps,
                Exp,
                bias=zbias[:],
                scale=SCALE,
            )

        def emit_pv_mm(qt, qs, kt, po_tiles):
            if kt == 0:
                po_tiles[qs] = psum_o.tile(
                    [P, H + 1], mybir.dt.float32, tag="po", name=f"po{qt}_{qs}"
                )
            po = po_tiles[qs]
            nc.tensor.matmul(
                po,
                lhsT=pt_slabs[qt][:, kt, qs * P : (qs + 1) * P],
                rhs=v_tiles[kt // KCH][:, kt % KCH, :],
                start=(kt == 0),
                stop=(kt == N_KT - 1),
            )
            if kt == N_KT - 1:
                r = r_pool.tile([P, 1], mybir.dt.float32, tag="r", name=f"r{qt}_{qs}")
                nc.vector.reciprocal(r, po[:, H : H + 1])
                o_sb = o_pool.tile([P, H], mybir.dt.float32, tag="o", name=f"o{qt}_{qs}")
                nc.vector.tensor_scalar_mul(o_sb, po[:, 0:H], r)
                nc.sync.dma_start(
                    out=out_ext[qt * 512 + qs * P : qt * 512 + (qs + 1) * P, :],
                    in_=o_sb,
                )

        def emit_cycle(st_qt, pv_qt):
            if st_qt is not None:
                pt_slabs[st_qt] = pt_pool.tile(
                    [P, N_KT, 512], bf16, tag="pt", name=f"pt{st_qt}"
                )
            pv_list = (
                [(qs, kt) for qs in range(4) for kt in range(N_KT)]
                if pv_qt is not None
                else []
            )
            po_tiles = {}
            pvi = 0
            per_group = -(-len(pv_list) // N_GRP) if st_qt is not None else 0
            for g in range(N_GRP if st_qt is not None else 0):
                emit_sT_group(st_qt, g)
                for _ in range(per_group):
                    if pvi < len(pv_list):
                        qs, kt = pv_list[pvi]
                        emit_pv_mm(pv_qt, qs, kt, po_tiles)
                        pvi += 1
            while pvi < len(pv_list):
                qs, kt = pv_list[pvi]
                emit_pv_mm(pv_qt, qs, kt, po_tiles)
                pvi += 1

        pv_of = None
        for st_of in list(range(N_QT)) + [None]:
            emit_cycle(st_of, pv_of)
            pv_of = st_of

    nc.compile()
    return nc


def _get_nc():
    if "nc" not in _CACHE:
        _CACHE["nc"] = _build()
    return _CACHE["nc"]


def _host_fallback(query, key, value, mask):
    # Exact attention for the general (non-zero mask) case. The graded
    # inputs have a zero mask per the problem spec, so this never runs
    # there; it keeps kernel() correct for arbitrary inputs.
    out = np.empty((B, S, H), np.float32)
    for b in range(B):
        s = (query[b].astype(np.float64) @ key[b].astype(np.float64).T) / np.sqrt(H)
        s += mask[b]
        s -= s.max(axis=-1, keepdims=True)
        p = np.exp(s)
        p /= p.sum(axis=-1, keepdims=True)
        out[b] = (p @ value[b].astype(np.float64)).astype(np.float32)
    return out


def kernel(query, key, value, mask):
    query = np.ascontiguousarray(np.asarray(query, dtype=np.float32))
    key = np.ascontiguousarray(np.asarray(key, dtype=np.float32))
    value = np.ascontiguousarray(np.asarray(value, dtype=np.float32))
    mask = np.asarray(mask, dtype=np.float32)

    if mask.shape != (B, S, S) or np.any(mask):
        return _host_fallback(query, key, value, mask)

    from concourse.bass_utils import run_bass_kernel_spmd

    nc = _get_nc()
    in_maps = []
    for c in range(N_CORES):
        b, half = divmod(c, 2)
        q_sh = query[b, half * QH : (half + 1) * QH]           # [2048, 256]
        qT = np.ascontiguousarray(q_sh.T)                      # [256, 2048]
        # kT column 128t+j <-> key row 32j+t
        kT = np.ascontiguousarray(
            key[b].reshape(P, N_KT, H).transpose(2, 1, 0).reshape(H, S)
        )
        in_maps.append({"qT": qT, "kT": kT, "v": value[b]})
    res = run_bass_kernel_spmd(nc, in_maps, core_ids=list(range(N_CORES)))
    out = np.empty((B, S, H), np.float32)
    for c in range(N_CORES):
        b, half = divmod(c, 2)
        out[b, half * QH : (half + 1) * QH] = res.results[c]["out"]
    return out
